# revision 1
# baseline (speedup 1.0000x reference)
"""CostVolume2D Trainium2 Bass kernel.

cost[n,d,h,w] = mean_c l[n,c,h,w] * r[n,c,h,w-d]  (0 for w < d)
N=8, C=32, H=256, W=512, D=64.  Data-parallel over batch: core i handles n=i.

Per-core algorithm (v-partition band correlation):
  For each row h and each v-block pair (2 blocks of 128 v each):
    M[v, w] = sum_c r[c,v] * l[c,w]  via TensorE matmuls (bf16, fp32 acc):
      stationary = r[c, wb:wb+128], moving = l[c, wb:wb+192]
    strip[p, d] = M[wb+p, n=p+d] = cost[d, h, wb+p+d]
  Strip extraction (a per-partition shear) rides a DRAM scratch round trip:
  the band tile [128, 384] is stored with flat-DRAM stride 447 per row
  (scratch[447*p + n] = band[p, n]), so strips become a rect gather
  ([[448,128],[192,2],[1,64]]).  Strips are transposed with the DMA xbar
  ([128,128] bf16) giving T[(k,d), p] = cost[d, h, wb_k+p+d], stored as
  contiguous 128-wide w-runs at flat stride HP*WOP+1.  The output tensor is
  padded [64, 257, 576] bf16: run-shift spill and w<d zero-store spill land
  in the padding, sliced off (and cast to f32) on the host.
"""

import numpy as np

_CACHE = {}

C, H, W, D = 32, 256, 512, 64
N_CORES = 8
WP = W + 64      # padded moving width
HP = H + 1       # padded out rows (absorbs h=0 zero-store spill)
WOP = W + 64     # padded out cols (absorbs w-run shift spill)
STILE = 127 * 447 + 384   # scratch elems per (h, wpair) tile


def _build(h_rows=H):
    import concourse.tile as tile
    from concourse import bacc, mybir
    from concourse.ap import AP

    f32 = mybir.dt.float32
    bf16 = mybir.dt.bfloat16

    nc = bacc.Bacc("TRN2", target_bir_lowering=False, debug=False)
    l_d = nc.dram_tensor("l", [C, h_rows, W], f32, kind="ExternalInput")
    r_d = nc.dram_tensor("r", [C, h_rows, W], f32, kind="ExternalInput")
    o_d = nc.dram_tensor("o", [1, D * (h_rows + 1) * WOP], bf16,
                         kind="ExternalOutput")
    scr = nc.dram_tensor("scr", [1, 2 * h_rows * STILE], bf16, kind="Internal")
    HPWOP = (h_rows + 1) * WOP

    with tile.TileContext(nc) as tc:
        with (
            tc.tile_pool(name="io", bufs=4) as io_pool,
            tc.tile_pool(name="band", bufs=6) as band_pool,
            tc.tile_pool(name="xp", bufs=6) as xp_pool,
            tc.tile_pool(name="const", bufs=1) as const_pool,
            tc.tile_pool(name="psum", bufs=4, space="PSUM") as psum_pool,
        ):
            zero64 = const_pool.tile([64, 64], bf16)
            nc.gpsimd.memset(zero64[:], 0.0)

            for h in range(h_rows):
                lt = io_pool.tile([C, WP], bf16, tag="lt")
                nc.gpsimd.dma_start(lt[:, 0:W], l_d[:, h, :])  # f32->bf16 cast
                nc.gpsimd.memset(lt[:, W:WP], 0.0)
                rt = io_pool.tile([C, W], bf16, tag="rt")
                nc.gpsimd.dma_start(rt[:], r_d[:, h, :])

                # zeros for w < d of this row (spill -> previous row's pad)
                zdst = AP(o_d.ap().tensor, (1 + h) * WOP - 64,
                          [[HPWOP + 1, 64], [1, 64]])
                nc.sync.dma_start(zdst, zero64[:])

                for wpair in range(2):
                    psum2 = psum_pool.tile([128, 384], f32, tag="ps")
                    for k in range(2):
                        wb = (2 * wpair + k) * 128
                        nc.tensor.matmul(
                            psum2[:, 192 * k:192 * k + 192],
                            rt[:, wb:wb + 128],
                            lt[:, wb:wb + 192],
                            start=True, stop=True,
                        )
                    band = band_pool.tile([128, 384], bf16, tag="band")
                    if wpair == 0:
                        nc.vector.tensor_scalar_mul(band[:], psum2[:], 1.0 / C)
                    else:
                        nc.scalar.mul(band[:], psum2[:], 1.0 / C)

                    # sheared scratch write: scr[447*p + n] = band[p, n]
                    t = 2 * h + wpair
                    sw = AP(scr.ap().tensor, t * STILE, [[447, 128], [1, 384]])
                    (nc.sync if wpair == 0 else nc.scalar).dma_start(sw, band[:])

                    # rect strips gather: strips[p, (k,d)] = scr[448p+192k+d]
                    strips = xp_pool.tile([128, 128], bf16, tag="strips")
                    rd = AP(scr.ap().tensor, t * STILE,
                            [[448, 128], [192, 2], [1, 64]])
                    (nc.scalar if wpair == 0 else nc.sync).dma_start(strips[:], rd)

                    xt = xp_pool.tile([128, 128], bf16, tag="xt")
                    nc.sync.dma_start(xt[:], strips[:], transpose=True)

                    # store halves: out[d, 1+h, wb + d + p], p in [0,128)
                    for k in range(2):
                        sdst = AP(o_d.ap().tensor,
                                  (1 + h) * WOP + (2 * wpair + k) * 128,
                                  [[HPWOP + 1, 64], [1, 128]])
                        eng = nc.sync if k == 0 else nc.scalar
                        eng.dma_start(sdst, xt[64 * k:64 * k + 64, :])
    nc.compile()
    return nc


def _get_nc(h_rows=H):
    if h_rows not in _CACHE:
        _CACHE[h_rows] = _build(h_rows)
    return _CACHE[h_rows]


def kernel(l_fmap, r_fmap, use_naive, max_disp):
    from concourse.bass_utils import run_bass_kernel_spmd

    l_fmap = np.asarray(l_fmap, dtype=np.float32)
    r_fmap = np.asarray(r_fmap, dtype=np.float32)
    assert int(max_disp) == D, f"kernel hardcoded for max_disp={D}"
    n, c, h, w = l_fmap.shape
    assert (n, c, h, w) == (N_CORES, C, H, W)

    nc = _get_nc(H)
    in_maps = [
        {"l": np.ascontiguousarray(l_fmap[i]), "r": np.ascontiguousarray(r_fmap[i])}
        for i in range(N_CORES)
    ]
    res = run_bass_kernel_spmd(nc, in_maps, core_ids=list(range(N_CORES)))
    out = np.stack([
        np.asarray(res.results[i]["o"]).reshape(D, HP, WOP)[:, 1:, 0:W]
        for i in range(N_CORES)
    ])
    return out.astype(np.float32)



# revision 2
# speedup vs baseline: 1.0288x; 1.0288x over previous
"""CostVolume2D Trainium2 Bass kernel, v3 (DMA-batched + software-pipelined).

cost[n,d,h,w] = mean_c l[n,c,h,w] * r[n,c,h,w-d]  (0 for w < d)
N=8, C=32, H=256, W=512, D=64.  Data-parallel over batch: core i handles n=i.

The cost model charges ~630ns of shared HWDGE time plus ~600ns of issuing-
sequencer time PER DMA instruction; DMA transfer itself runs at
360GB/s (halved below 512B contiguous runs).  So: batch every DMA across
row-blocks of R=8 (7 HWDGE DMAs + 2 SWDGE loads per 8 rows) and
software-pipeline emission one block deep so in-order engine queues never
head-block on a same-block dependency (which would starve the DMA device).

Per block (rows hb..hb+R):
  loads    (Pool/SWDGE, prefetched 1 block ahead): l/r rows packed
           [C, R*512] bf16 (+64 tail pad; wb=384 moving windows read into
           the next row, producing values that land in output padding).
  front    per row h, wpair: band M[v,w]=sum_c r[c,v] l[c,w] via 2 TensorE
           matmuls (bf16, f32 psum) -> x(1/C) copy (Act/DVE) into
           band_blk [128, R*384] -> ONE sheared scratch write per wpair
           (SP): scr[(2h+wp)*STILE + 447p + n] = band[p, n].
  back     (emitted next iteration): 4 rect gathers (Act):
           G[p, 128h+64k+d] = scr[448p+192k+d]; per row+wpair one PE
           transpose-matmul against identity -> psumT[(k,d), p]; two
           partition-sliced copies (DVE/Pool) -> st_blk[d, 576h+64+256wp
           +128k+p]; cols [0,64) of each 576-row zeroed (strided memset);
           ONE store per block (SP): [[HPWOP+1,64],[WOP,R],[1,576]] at
           base (1+hb)*WOP-64 -> element j lands at out[d,1+h,-64+d+j],
           covering the w<d zero region and both wpairs in 1152B runs.

Output padded [64, 257, 576] bf16 (row 0 / cols >= 512 absorb spill),
sliced + cast to f32 on host.  Inputs host-cast to bf16 (halves HBM reads).
"""

import numpy as np

_CACHE = {}

C, H, W, D = 32, 256, 512, 64
N_CORES = 8
R = 8                    # rows per block
HP = H + 1               # padded out rows (absorbs h=0 / w<d spill)
WOP = W + 64             # padded out cols (absorbs w-run shift spill)
SROW = W + 64            # st block row width: 64 zeros + 2*256 data
STILE = 127 * 447 + 384  # scratch elems per (h, wpair) tile


def _build(h_rows=H):
    import concourse.tile as tile
    from concourse import bacc, masks, mybir
    from concourse.ap import AP

    f32 = mybir.dt.float32
    bf16 = mybir.dt.bfloat16

    assert h_rows % R == 0
    nc = bacc.Bacc("TRN2", target_bir_lowering=False, debug=False)
    l_d = nc.dram_tensor("l", [C, h_rows, W], bf16, kind="ExternalInput")
    r_d = nc.dram_tensor("r", [C, h_rows, W], bf16, kind="ExternalInput")
    o_d = nc.dram_tensor("o", [1, D * (h_rows + 1) * WOP], bf16,
                         kind="ExternalOutput")
    scr = nc.dram_tensor("scr", [1, 2 * h_rows * STILE], bf16, kind="Internal")
    HPWOP = (h_rows + 1) * WOP

    with tile.TileContext(nc) as tc:
        with (
            tc.tile_pool(name="io", bufs=3) as io_pool,
            tc.tile_pool(name="band", bufs=3) as band_pool,
            tc.tile_pool(name="gat", bufs=3) as g_pool,
            tc.tile_pool(name="st", bufs=3) as st_pool,
            tc.tile_pool(name="const", bufs=1) as const_pool,
            tc.tile_pool(name="psum", bufs=5, space="PSUM") as psum_pool,
            tc.tile_pool(name="psumT", bufs=3, space="PSUM") as psumT_pool,
        ):
            ident = const_pool.tile([128, 128], bf16)
            masks.make_identity(nc, ident[:])

            def emit_loads(hb):
                lt = io_pool.tile([C, R * W + 64], bf16, tag="lt")
                nc.gpsimd.dma_start(lt[:, 0:R * W], l_d[:, hb:hb + R, :])
                nc.gpsimd.memset(lt[:, R * W:R * W + 64], 0.0)
                rt = io_pool.tile([C, R * W], bf16, tag="rt")
                nc.gpsimd.dma_start(rt[:], r_d[:, hb:hb + R, :])
                return lt, rt

            def emit_front(hb, lt, rt):
                bands = [band_pool.tile([128, R * 384], bf16, tag=f"b{w}",
                                        name=f"band{w}") for w in range(2)]
                for hh in range(R):
                    for wpair in range(2):
                        psum2 = psum_pool.tile([128, 384], f32, tag="ps")
                        for k in range(2):
                            wb = (2 * wpair + k) * 128
                            nc.tensor.matmul(
                                psum2[:, 192 * k:192 * k + 192],
                                rt[:, hh * W + wb:hh * W + wb + 128],
                                lt[:, hh * W + wb:hh * W + wb + 192],
                                start=True, stop=True,
                            )
                        dst = bands[wpair][:, hh * 384:(hh + 1) * 384]
                        if (hh + wpair) % 2 == 0:
                            nc.vector.tensor_scalar_mul(dst, psum2[:], 1.0 / C)
                        else:
                            nc.scalar.mul(dst, psum2[:], 1.0 / C)
                # sheared scratch write, one DMA per wpair:
                #   scr[(2h+wp)*STILE + 447p + n] = band[p, n]
                for wpair in range(2):
                    sw = AP(scr.ap().tensor, (2 * hb + wpair) * STILE,
                            [[447, 128], [2 * STILE, R], [1, 384]])
                    nc.sync.dma_start(sw, bands[wpair][:])

            def emit_gathers(hb):
                # G[wp][k][p, 64h + d] = scr[(2h+wp)*STILE + 448p + 192k + d]
                gts = []
                for wpair in range(2):
                    row = []
                    for k in range(2):
                        gt = g_pool.tile([128, R * 64], bf16,
                                         tag=f"g{wpair}{k}",
                                         name=f"g{wpair}{k}")
                        row.append(gt)
                        rd = AP(scr.ap().tensor,
                                (2 * hb + wpair) * STILE + 192 * k,
                                [[448, 128], [2 * STILE, R], [1, 64]])
                        nc.sync.dma_start(gt[:], rd)
                    gts.append(row)
                return gts

            def emit_back(hb, gts):
                stb = st_pool.tile([64, R * SROW], bf16, tag="stb")
                for hh in range(R):
                    nc.gpsimd.memset(stb[:, hh * SROW:hh * SROW + 64], 0.0)
                # two transposes G-half -> psumT[d, 128k+p]; one wide copy
                # lands it in the st block as [d, 576h + 64 + 256wp + 128k + p]
                idx = 0
                for hh in range(R):
                    for wpair in range(2):
                        psumT = psumT_pool.tile([64, 256], bf16, tag="pt")
                        for k in range(2):
                            nc.tensor.matmul(
                                psumT[:, 128 * k:128 * k + 128],
                                gts[wpair][k][:, hh * 64:hh * 64 + 64],
                                ident[:],
                                start=True, stop=True,
                                is_transpose=True,
                            )
                        base = hh * SROW + 64 + 256 * wpair
                        dst = stb[:, base:base + 256]
                        if idx % 4 == 3:
                            nc.scalar.copy(dst, psumT[:])
                        else:
                            nc.vector.tensor_copy(dst, psumT[:])
                        idx += 1
                # one store per block: out[d, 1+hb+h, -64+d+j] = stb[d, 576h+j]
                sdst = AP(o_d.ap().tensor, (1 + hb) * WOP - 64,
                          [[HPWOP + 1, 64], [WOP, R], [1, SROW]])
                nc.sync.dma_start(sdst, stb[:])

            # 2-deep software pipeline.  Iteration i emits: back half of
            # block i-2 (its gathers were issued at the end of iteration i-1,
            # so every dep is old), load prefetch for block i+1, front of
            # block i, and finally gathers for block i-1 (their producer
            # scratch-write completes early in this iteration, and their
            # consumers run next iteration).
            blocks = list(range(0, h_rows, R))
            io_tiles = {blocks[0]: emit_loads(blocks[0])}
            gts_map = {}
            for i, hb in enumerate(blocks):
                if i >= 2:
                    emit_back(blocks[i - 2], gts_map.pop(blocks[i - 2]))
                if i + 1 < len(blocks):
                    io_tiles[blocks[i + 1]] = emit_loads(blocks[i + 1])
                lt, rt = io_tiles.pop(hb)
                emit_front(hb, lt, rt)
                if i >= 1:
                    gts_map[blocks[i - 1]] = emit_gathers(blocks[i - 1])
            for i in (len(blocks) - 2, len(blocks) - 1):
                if i >= 0:
                    if blocks[i] not in gts_map:
                        gts_map[blocks[i]] = emit_gathers(blocks[i])
                    emit_back(blocks[i], gts_map.pop(blocks[i]))
    nc.compile()
    return nc


def _get_nc(h_rows=H):
    if h_rows not in _CACHE:
        _CACHE[h_rows] = _build(h_rows)
    return _CACHE[h_rows]


def kernel(l_fmap, r_fmap, use_naive, max_disp):
    import ml_dtypes
    from concourse.bass_utils import run_bass_kernel_spmd

    assert int(max_disp) == D, f"kernel hardcoded for max_disp={D}"
    bf = ml_dtypes.bfloat16
    l_fmap = np.asarray(l_fmap, dtype=np.float32).astype(bf)
    r_fmap = np.asarray(r_fmap, dtype=np.float32).astype(bf)
    n, c, h, w = l_fmap.shape
    assert (n, c, h, w) == (N_CORES, C, H, W)

    nc = _get_nc(H)
    in_maps = [
        {"l": np.ascontiguousarray(l_fmap[i]), "r": np.ascontiguousarray(r_fmap[i])}
        for i in range(N_CORES)
    ]
    res = run_bass_kernel_spmd(nc, in_maps, core_ids=list(range(N_CORES)))
    out = np.stack([
        np.asarray(res.results[i]["o"]).reshape(D, HP, WOP)[:, 1:, 0:W]
        for i in range(N_CORES)
    ])
    return out.astype(np.float32)
